# revision 31
# baseline (speedup 1.0000x reference)
"""Packed causal GQA attention (B=4 x S=1024, H=32, KVH=8, D=DV=128, fp32)
for 8 Trainium2 NeuronCores.

Sharding: tensor-parallel over KV heads. Core c owns kv head c and its GQA
group of 4 query heads (4c..4c+3). No cross-core communication. As part of
the host-side shard/layout glue, Q and K are pre-transposed to [d, t] and
cast to fp16 (fp16 round-off ~2.4e-4 relative, matching the overall error
budget); V is cast to fp16. The kernel output is per-head-transposed
out^T[dv, q] plus implicit normalization; the host transposes back while
unsharding.

Per-core pipeline, software-pipelined over 16 (b, h) units:
  - Per (b,h,kb): S^T[k, q] = K^T.T @ Q^T on PE (fp16 in, fp32 PSUM), causal
    column ranges only; P^T = Exp(SCALE*S^T) on ACT -> fp16 tiles; the
    strictly-upper triangle of each diagonal block is zeroed by a gpsimd
    affine_select.
  - out^T[dv, q] = sum_kb V[kb].T @ P^T[kb], l[q] = sum_kb 1.T @ P^T[kb]
    (fp16 matmuls, fp32 PSUM accumulation; the ones-matmul broadcasts the
    softmax denominator over all 128 partitions).
  - out = out^T * (1/l) via DVE reciprocal_approx_fast + multiply.

All DMAs are plain HWDGE loads/stores (no DMA-transposes, no SWDGE casts):
mixing HWDGE transposes with other DMA traffic serializes on xbar-mode
transitions and corrupts concurrent plain copies, so we avoid the xbar
entirely.
"""

import os
from collections import deque

import numpy as np

import concourse.bacc as bacc
import concourse.tile as tile
from concourse import mybir, bass_utils

T = 4096          # packed tokens
SEQ = 1024        # per-sequence length
B = T // SEQ      # 4 sequences
H = 32            # query heads (total)
KVH = 8           # kv heads (total)
D = 128           # head size
DV = 128          # value head size
NCORES = 8
HPC = H // NCORES         # 4 query heads per core
NB = SEQ // 128           # 8 k-blocks per sequence
SCALE = 0.08838834764831845

F16 = mybir.dt.float16
F32 = mybir.dt.float32

_BUILD_CACHE = {}


def _build_nc():
    nc = bacc.Bacc("TRN2", target_bir_lowering=False, debug=False,
                   num_devices=NCORES)
    # host-pretransposed, fp16: qT[h*128+d, t], kT[d, t], v[t, dv]
    qt_dram = nc.dram_tensor("qT", [HPC * D, T], F16, kind="ExternalInput").ap()
    kt_dram = nc.dram_tensor("kT", [D, T], F16, kind="ExternalInput").ap()
    v_dram = nc.dram_tensor("v", [T, DV], F16, kind="ExternalInput").ap()
    # out_t[b*HPC + h, dv, q]  (transposed per-head output; host untransposes)
    out_dram = nc.dram_tensor("out_t", [B * HPC, DV, SEQ], F16,
                              kind="ExternalOutput").ap()

    with tile.TileContext(nc) as tc:
        with tc.tile_pool(name="consts", bufs=1) as consts, \
             tc.tile_pool(name="kv", bufs=2) as kv_pool, \
             tc.tile_pool(name="qts", bufs=5) as qt_pool, \
             tc.tile_pool(name="pt", bufs=4) as pt_pool, \
             tc.tile_pool(name="work", bufs=2) as work, \
             tc.tile_pool(name="pp_s", bufs=2, space="PSUM") as pp_s, \
             tc.tile_pool(name="pp_ol", bufs=4, space="PSUM") as pp_ol:

            ones_sb = consts.tile([128, 512], F16, tag="ones")
            nc.vector.memset(ones_sb[:], 1.0)

            # HAM clock warmup: a few dependency-free matmuls that run while
            # the first input chunks are still in flight, so the PE clock
            # gate starts ramping toward 2.4 GHz before real work issues.
            # Allocated from pp_ol: its slots are first recycled deep into
            # back(1), well after the last interleaved warmup fires.
            warm_ps = pp_ol.tile([128, 512], F32, tag="ps_ol")
            for _ in range(4):
                nc.tensor.matmul(warm_ps[:, 0:512], ones_sb[:, 0:128],
                                 ones_sb[:], start=True, stop=True,
                                 skip_group_check=True)

            per_b = {}   # b -> (kt, v_sb, [qt0..qt3])

            def emit_loads(b, chunks=1):
                """Load kt + qt0; chunks>1 splits them into kb-block chunks
                issued high-blocks-first so a descending-kb front can start
                after only the first chunk lands."""
                cols = slice(b * SEQ, (b + 1) * SEQ)
                kt = kv_pool.tile([128, NB, 128], F16, tag="kt")
                qt = qt_pool.tile([128, NB, 128], F16, tag="qt")
                step = NB // chunks
                for c in range(chunks - 1, -1, -1):
                    bs, be = c * step, (c + 1) * step
                    ccols = slice(b * SEQ + bs * 128, b * SEQ + be * 128)
                    nc.sync.dma_start(
                        kt[:, bs:be],
                        kt_dram[:, ccols].rearrange("d (nb t) -> d nb t", t=128))
                    nc.sync.dma_start(
                        qt[:, bs:be],
                        qt_dram[0:D, ccols].rearrange("d (nb t) -> d nb t", t=128))
                per_b[b] = (kt, None, [qt])

            def emit_late_loads(b, h):
                """After front(b, h) is emitted: pull in the next tensors."""
                cols = slice(b * SEQ, (b + 1) * SEQ)
                rows = slice(b * SEQ, (b + 1) * SEQ)
                kt, v_sb, qts = per_b[b]
                if h + 1 < HPC:
                    qt = qt_pool.tile([128, NB, 128], F16, tag="qt")
                    nc.sync.dma_start(
                        qt[:],
                        qt_dram[(h + 1) * D:(h + 2) * D, cols].rearrange(
                            "d (nb t) -> d nb t", t=128))
                    qts.append(qt)
                if h == 0:
                    v_sb = kv_pool.tile([128, NB, DV], F16, tag="v")
                    nc.sync.dma_start(
                        v_sb[:],
                        v_dram[rows, :].rearrange("(nb p) d -> p nb d", p=128))
                per_b[b] = (kt, v_sb, qts)

            def emit_front(b, h, descending=False):
                """QK matmuls + exp + causal mask -> dict kb -> P^T tile.

                descending=True runs kb 7..0 so the first matmuls only need
                the high kt/qt blocks (which chunked loads deliver first)."""
                kt, _, qts = per_b[b]
                qt = qts[h]
                pts = {}
                order = range(NB - 1, -1, -1) if descending else range(NB)
                for ikb, kb in enumerate(order):
                    if descending and ikb < 4:
                        # keep the HAM activity window dense while the later
                        # input chunks are still in flight (clock warmup)
                        for _ in range(3):
                            nc.tensor.matmul(
                                warm_ps[:, 0:512], ones_sb[:, 0:128],
                                ones_sb[:], start=True, stop=True,
                                skip_group_check=True)
                    ncols_t = SEQ - 128 * kb
                    pt = pt_pool.tile([128, ncols_t], F16, tag=f"pt{kb}")
                    # [128, 1024] psum tile (2 banks); kb>=4 uses cols 512:
                    ps = pp_s.tile([128, 1024], F32, tag="ps_s")
                    for qc in range(kb // 4, 2):
                        qs = max(128 * kb, 512 * qc)
                        qe = 512 * (qc + 1)
                        nc.tensor.matmul(
                            ps[:, qs:qe],
                            kt[:, kb, :],
                            qt[:, qs // 128:qe // 128, :],
                            start=True, stop=True, skip_group_check=True)
                    nc.scalar.activation(
                        pt[:], ps[:, 128 * kb:],
                        mybir.ActivationFunctionType.Exp, scale=SCALE)
                    # zero strictly-upper triangle of the diagonal block
                    nc.gpsimd.affine_select(
                        out=pt[:, 0:128], in_=pt[:, 0:128],
                        compare_op=mybir.AluOpType.is_ge,
                        fill=0.0, base=0,
                        pattern=[[1, 128]], channel_multiplier=-1)
                    pts[kb] = pt
                return pts

            def emit_back(b, h, pts):
                """PV + denominator matmuls, normalize, store."""
                _, v_sb, _ = per_b[b]
                out_sb = work.tile([128, SEQ], F16, tag="out_sb")
                for qc in range(2):
                    kbs = list(range(0, 4 * qc + 4))
                    ps_o = pp_ol.tile([128, 512], F32, tag="ps_ol")
                    ps_l = pp_ol.tile([128, 512], F32, tag="ps_ol")
                    for kb in kbs:
                        qs = max(128 * kb, 512 * qc)
                        qe = 512 * (qc + 1)
                        rhs = pts[kb][:, qs - 128 * kb:qe - 128 * kb]
                        flags = dict(start=(kb == 0), stop=(kb == kbs[-1]),
                                     skip_group_check=True)
                        # denominator group first so the DVE reciprocal can
                        # overlap the PV matmul stream
                        nc.tensor.matmul(
                            ps_l[:, qs - 512 * qc:512], ones_sb[:, 0:128], rhs,
                            **flags)
                        nc.tensor.matmul(
                            ps_o[:, qs - 512 * qc:512],
                            v_sb[:, kb, :], rhs, **flags)
                    rsb = work.tile([128, 512], F32, tag="rsb")
                    nc.vector.reciprocal_approx_fast(rsb[:], ps_l[:])
                    nc.vector.tensor_tensor(
                        out=out_sb[:, qc * 512:(qc + 1) * 512],
                        in0=ps_o[:], in1=rsb[:], op=mybir.AluOpType.mult)
                nc.sync.dma_start(out_dram[b * HPC + h], out_sb[:])

            # software-pipelined emission at depth 2: front(u+1) and
            # front(u+2) are emitted before back(u), so the exp/mask chain
            # of a unit has two units' worth of PE work to hide behind.
            units = [(b, h) for b in range(B) for h in range(HPC)]
            pending = deque()   # (b, h, pts)
            for b, h in units:
                if h == 0:
                    emit_loads(b, chunks=(2 if b == 0 else 1))
                pts = emit_front(b, h, descending=(b == 0 and h == 0))
                emit_late_loads(b, h)
                pending.append((b, h, pts))
                if len(pending) > 2:
                    emit_back(*pending.popleft())
            while pending:
                emit_back(*pending.popleft())

    nc.compile()
    return nc


# ---------------------------------------------------------------------------
# Phase B: grouped-exp restructure.
#
# Dropping the per-kb denominator ones-matmuls (24us of PE) in favour of a
# DVE collapse of the P^T tiles + one accumulating ones-matmul per q-half,
# and cutting the 128 per-kb exp ACTIVATEs (253ns fixed overhead each) to 48
# by packing each unit's causal score blocks into three [128,1536] PSUM
# tiles, exactly (zero padding):
#   g0: kb0 @ 0    (w 1024), kb4 @ 1024 (w 512)
#   g1: kb1 @ 0    (w 896),  kb3 @ 896  (w 640)
#   g2: kb2 @ 0    (w 768),  kb5 @ 768 (w 384), kb6 @ 1152 (w 256),
#       kb7 @ 1408 (w 128)
# Every matmul chunk is split at absolute 512-col PSUM bank boundaries and
# every chunk's q-start is 128-aligned. PSUM: 2x[128,1536] qk (6 banks) +
# [128,512] l (1 bank) + [128,512] out (1 bank) = 8 banks.
# ---------------------------------------------------------------------------

GROUPS = [
    [(0, 0), (4, 1024)],
    [(1, 0), (3, 896)],
    [(2, 0), (5, 768), (6, 1152), (7, 1408)],
]
# Unit-0 prologue table: groups ordered so each becomes runnable as its
# descending load chunk lands — {kb6,kb7} needs only blocks 6-7 (first 64KB
# chunk), {kb4,kb5} blocks 4-7, etc. The exp pipeline starts ~4us earlier
# than with the steady-state table.
GROUPS0 = [
    [(6, 0), (7, 256)],
    [(4, 0), (5, 512)],
    [(2, 0), (3, 768)],
    [(1, 0)],
    [(0, 0)],
]
LOAD_SPLITS0 = [(6, 8), (4, 6), (2, 4), (0, 2)]


def _loc(table):
    return {kb: (g, off) for g, lst in enumerate(table) for kb, off in lst}


def _qk_chunks(kb, off):
    """Sub-chunks (tile_c0, tile_c1, q0, q1) of kb's packed score region,
    split at absolute 512-col boundaries."""
    w = 1024 - 128 * kb
    out = []
    c = off
    while c < off + w:
        ce = min(off + w, (c // 512 + 1) * 512)
        q0 = 128 * kb + (c - off)
        out.append((c, ce, q0, q0 + (ce - c)))
        c = ce
    return out


def _pv_slices(qc, loc):
    """(kb, group, tile_c0, tile_c1, out_c0) for PV accumulation of q-half."""
    out = []
    for kb in range(8):
        g, off = loc[kb]
        q0 = max(128 * kb, 512 * qc)
        q1 = 512 * (qc + 1)
        if q0 >= q1:
            continue
        out.append((kb, g, off + (q0 - 128 * kb), off + (q1 - 128 * kb),
                    q0 - 512 * qc))
    return out


def _build_nc_b():
    nc = bacc.Bacc("TRN2", target_bir_lowering=False, debug=False,
                   num_devices=NCORES)
    qt_dram = nc.dram_tensor("qT", [HPC * D, T], F16, kind="ExternalInput").ap()
    kt_dram = nc.dram_tensor("kT", [D, T], F16, kind="ExternalInput").ap()
    v_dram = nc.dram_tensor("v", [T, DV], F16, kind="ExternalInput").ap()
    out_dram = nc.dram_tensor("out_t", [B * HPC, DV, SEQ], F16,
                              kind="ExternalOutput").ap()

    ADD = mybir.AluOpType.add
    MUL = mybir.AluOpType.mult
    EXP = mybir.ActivationFunctionType.Exp

    with tile.TileContext(nc) as tc:
        with tc.tile_pool(name="consts", bufs=1) as consts, \
             tc.tile_pool(name="kv", bufs=2) as kv_pool, \
             tc.tile_pool(name="qts", bufs=5) as qt_pool, \
             tc.tile_pool(name="pt", bufs=3) as pt_pool, \
             tc.tile_pool(name="ls", bufs=3) as ls_pool, \
             tc.tile_pool(name="work", bufs=2) as work, \
             tc.tile_pool(name="pp_qk", bufs=2, space="PSUM") as pp_qk, \
             tc.tile_pool(name="pp_l", bufs=1, space="PSUM") as pp_l, \
             tc.tile_pool(name="pp_o", bufs=1, space="PSUM") as pp_o:

            ones_sb = consts.tile([128, 512], F16, tag="ones")
            nc.vector.memset(ones_sb[:], 1.0)

            # HAM clock warmup (pp_l's slot is first reused by back1(0),
            # long after the last warmup matmul fires)
            warm_ps = pp_l.tile([128, 512], F32, tag="pl")
            for _ in range(4):
                nc.tensor.matmul(warm_ps[:], ones_sb[:, 0:128], ones_sb[:],
                                 start=True, stop=True, skip_group_check=True)

            per_b = {}

            def emit_loads(b, splits=((0, NB),)):
                """Each dma_start lands on its own DMA queue; descending
                block splits let the unit-0 per-group front start as soon as
                the first small chunk arrives."""
                kt = kv_pool.tile([128, NB, 128], F16, tag="kt")
                qt = qt_pool.tile([128, NB, 128], F16, tag="qt")
                for bs, be in splits:
                    ccols = slice(b * SEQ + bs * 128, b * SEQ + be * 128)
                    nc.sync.dma_start(
                        kt[:, bs:be],
                        kt_dram[:, ccols].rearrange("d (nb t) -> d nb t",
                                                    t=128))
                    nc.sync.dma_start(
                        qt[:, bs:be],
                        qt_dram[0:D, ccols].rearrange("d (nb t) -> d nb t",
                                                      t=128))
                per_b[b] = [kt, None, [qt]]

            def emit_late_loads(b, h):
                cols = slice(b * SEQ, (b + 1) * SEQ)
                rows = slice(b * SEQ, (b + 1) * SEQ)
                kt, v_sb, qts = per_b[b]
                if h + 1 < HPC:
                    qt = qt_pool.tile([128, NB, 128], F16, tag="qt")
                    nc.sync.dma_start(
                        qt[:],
                        qt_dram[(h + 1) * D:(h + 2) * D, cols].rearrange(
                            "d (nb t) -> d nb t", t=128))
                    qts.append(qt)
                if h == 0:
                    v_sb = kv_pool.tile([128, NB, DV], F16, tag="v")
                    nc.sync.dma_start(
                        v_sb[:],
                        v_dram[rows, :].rearrange("(nb p) d -> p nb d", p=128))
                per_b[b] = [kt, v_sb, qts]

            def emit_group(b, h, table, g, warm=False):
                """QK matmuls for one packed group + one wide exp + masks."""
                kt, _, qts = per_b[b]
                qt = qts[h]
                gps = pp_qk.tile([128, 1536], F32, tag="qk")
                for kb, off in table[g]:
                    if warm:
                        for _ in range(2):
                            nc.tensor.matmul(
                                warm_ps[:], ones_sb[:, 0:128], ones_sb[:],
                                start=True, stop=True, skip_group_check=True)
                    for c0, c1, q0, q1 in _qk_chunks(kb, off):
                        nc.tensor.matmul(
                            gps[:, c0:c1], kt[:, kb, :],
                            qt[:, q0 // 128:q1 // 128, :],
                            start=True, stop=True, skip_group_check=True)
                wmax = max(off + 1024 - 128 * kb for kb, off in table[g])
                gpt = pt_pool.tile([128, 1536], F16, tag=f"g{g}")
                nc.scalar.activation(gpt[:, 0:wmax], gps[:, 0:wmax], EXP,
                                     scale=SCALE)
                for kb, off in table[g]:
                    nc.gpsimd.affine_select(
                        out=gpt[:, off:off + 128], in_=gpt[:, off:off + 128],
                        compare_op=mybir.AluOpType.is_ge,
                        fill=0.0, base=0,
                        pattern=[[1, 128]], channel_multiplier=-1)
                return gpt

            def emit_collapse(loc, gpts):
                """lsum[k, q] = sum of P^T over all 8 kb."""
                ls = ls_pool.tile([128, 1024], F16, tag="ls")
                tt = nc.vector.tensor_tensor

                def sl(kb, a, bb):
                    g, off = loc[kb]
                    return gpts[g][:, off + a:off + bb]

                tt(out=ls[:, 128:1024], in0=sl(0, 128, 1024),
                   in1=sl(1, 0, 896), op=ADD)                     # kb0+kb1
                nc.vector.tensor_scalar_add(ls[:, 0:128], sl(0, 0, 128), 0.0)
                for kb in (2, 3, 5, 6, 7):
                    w = 1024 - 128 * kb
                    tt(out=ls[:, 128 * kb:1024], in0=ls[:, 128 * kb:1024],
                       in1=sl(kb, 0, w), op=ADD)
                # kb4's add runs on gpsimd (SBUF-only op) to relieve DVE
                nc.gpsimd.tensor_tensor(
                    out=ls[:, 512:1024], in0=ls[:, 512:1024],
                    in1=sl(4, 0, 512), op=ADD)
                return ls

            def emit_back1(rec):
                """qc0: denominator ones-matmul, reciprocal, PV, normalize."""
                b, h, loc, gpts, ls = rec
                _, v_sb, _ = per_b[b]
                pl = pp_l.tile([128, 512], F32, tag="pl")
                nc.tensor.matmul(pl[:], ones_sb[:, 0:128], ls[:, 0:512],
                                 start=True, stop=True, skip_group_check=True)
                rs = work.tile([128, 512], F32, tag="rs")
                nc.vector.reciprocal_approx_fast(rs[:], pl[:])
                po = pp_o.tile([128, 512], F32, tag="po")
                sl = _pv_slices(0, loc)
                for i, (kb, g, c0, c1, oc) in enumerate(sl):
                    nc.tensor.matmul(
                        po[:, oc:512], v_sb[:, kb, :], gpts[g][:, c0:c1],
                        start=(i == 0), stop=(i == len(sl) - 1),
                        skip_group_check=True)
                os = work.tile([128, SEQ], F16, tag="out_sb")
                nc.vector.tensor_tensor(out=os[:, 0:512], in0=po[:],
                                        in1=rs[:], op=MUL)
                nc.sync.dma_start(out_dram[b * HPC + h][:, 0:512],
                                  os[:, 0:512])
                rec.append(os)

            def emit_back2(rec):
                """qc1."""
                b, h, loc, gpts, ls, os = rec
                _, v_sb, _ = per_b[b]
                pl = pp_l.tile([128, 512], F32, tag="pl")
                nc.tensor.matmul(pl[:], ones_sb[:, 0:128], ls[:, 512:1024],
                                 start=True, stop=True, skip_group_check=True)
                rs = work.tile([128, 512], F32, tag="rs")
                nc.vector.reciprocal_approx_fast(rs[:], pl[:])
                po = pp_o.tile([128, 512], F32, tag="po")
                sl = _pv_slices(1, loc)
                for i, (kb, g, c0, c1, oc) in enumerate(sl):
                    nc.tensor.matmul(
                        po[:, oc:512], v_sb[:, kb, :], gpts[g][:, c0:c1],
                        start=(i == 0), stop=(i == len(sl) - 1),
                        skip_group_check=True)
                nc.vector.tensor_tensor(out=os[:, 512:1024], in0=po[:],
                                        in1=rs[:], op=MUL)
                nc.sync.dma_start(out_dram[b * HPC + h][:, 512:1024],
                                  os[:, 512:1024])

            # Software pipeline at depth 2: back1(u-2) lands between the
            # current unit's groups, back2(u-2) after its collapse, keeping
            # each unit's reciprocal two units ahead of the PE work that
            # consumes it in the in-order DVE queue.
            units = [(b, h) for b in range(B) for h in range(HPC)]
            pending = deque()
            for b, h in units:
                if h == 0:
                    emit_loads(b, splits=(LOAD_SPLITS0 if b == 0
                                          else ((0, NB),)))
                warm = (b == 0 and h == 0)
                table = GROUPS0 if warm else GROUPS
                loc = _loc(table)
                gpts = [emit_group(b, h, table, 0, warm=warm),
                        emit_group(b, h, table, 1, warm=warm)]
                if len(pending) >= 2:
                    emit_back1(pending[0])
                for g in range(2, len(table)):
                    gpts.append(emit_group(b, h, table, g, warm=warm))
                ls = emit_collapse(loc, gpts)
                emit_late_loads(b, h)
                if len(pending) >= 2:
                    emit_back2(pending.popleft())
                pending.append([b, h, loc, gpts, ls])
            while pending:
                rec = pending.popleft()
                emit_back1(rec)
                emit_back2(rec)

    nc.compile()
    return nc


def run_sharded(query, key, value, trace=False):
    """Shard over 8 cores, run the bass kernel, unshard. Returns
    (out [T, H*DV] fp32, BassKernelResults)."""
    query = np.asarray(query, dtype=np.float32)
    key = np.asarray(key, dtype=np.float32)
    value = np.asarray(value, dtype=np.float32)

    ver = os.environ.get("ATTN_KERNEL_VER", "b")
    if ("nc", ver) not in _BUILD_CACHE:
        _BUILD_CACHE[("nc", ver)] = _build_nc_b() if ver == "b" else _build_nc()
    nc = _BUILD_CACHE[("nc", ver)]

    # host layout glue: cast to fp16, then transpose to [d, t]
    qT = np.ascontiguousarray(query.astype(np.float16).T)   # [H*D, T]
    kT = np.ascontiguousarray(key.astype(np.float16).T)     # [KVH*D, T]
    v16 = np.ascontiguousarray(value.astype(np.float16))    # [T, KVH*DV]

    in_maps = []
    for c in range(NCORES):
        in_maps.append({
            "qT": np.ascontiguousarray(qT[c * HPC * D:(c + 1) * HPC * D]),
            "kT": np.ascontiguousarray(kT[c * D:(c + 1) * D]),
            "v": np.ascontiguousarray(v16[:, c * DV:(c + 1) * DV]),
        })

    res = bass_utils.run_bass_kernel_spmd(
        nc, in_maps, core_ids=list(range(NCORES)), trace=trace)

    outs = []
    for c in range(NCORES):
        ot = res.results[c]["out_t"].astype(np.float32)  # [B*HPC, DV, SEQ] f16
        o = ot.reshape(B, HPC, DV, SEQ).transpose(0, 3, 1, 2).reshape(T, HPC * DV)
        outs.append(o)
    return np.concatenate(outs, axis=1), res


def kernel(query, key, value, seq_len=1024, **_unused):
    assert int(seq_len) == SEQ, f"kernel hardcodes seq_len={SEQ}, got {seq_len}"
    out, _ = run_sharded(query, key, value, trace=False)
    return out



# revision 39
# speedup vs baseline: 1.1119x; 1.1119x over previous
"""Packed causal GQA attention (B=4 x S=1024, H=32, KVH=8, D=DV=128, fp32)
for 8 Trainium2 NeuronCores.

Sharding: tensor-parallel over KV heads. Core c owns kv head c and its GQA
group of 4 query heads (4c..4c+3). No cross-core communication. As part of
the host-side shard/layout glue, Q and K are pre-transposed to [d, t] and
cast to fp16 (fp16 round-off ~2.4e-4 relative, matching the overall error
budget); V is cast to fp16. The kernel output is per-head-transposed
out^T[dv, q] plus implicit normalization; the host transposes back while
unsharding.

Per-core pipeline, software-pipelined over 16 (b, h) units:
  - Per (b,h,kb): S^T[k, q] = K^T.T @ Q^T on PE (fp16 in, fp32 PSUM), causal
    column ranges only; P^T = Exp(SCALE*S^T) on ACT -> fp16 tiles; the
    strictly-upper triangle of each diagonal block is zeroed by a gpsimd
    affine_select.
  - out^T[dv, q] = sum_kb V[kb].T @ P^T[kb], l[q] = sum_kb 1.T @ P^T[kb]
    (fp16 matmuls, fp32 PSUM accumulation; the ones-matmul broadcasts the
    softmax denominator over all 128 partitions).
  - out = out^T * (1/l) via DVE reciprocal_approx_fast + multiply.

All DMAs are plain HWDGE loads/stores (no DMA-transposes, no SWDGE casts):
mixing HWDGE transposes with other DMA traffic serializes on xbar-mode
transitions and corrupts concurrent plain copies, so we avoid the xbar
entirely.
"""

import os
from collections import deque

import numpy as np

import concourse.bacc as bacc
import concourse.tile as tile
from concourse import mybir, bass_utils

T = 4096          # packed tokens
SEQ = 1024        # per-sequence length
B = T // SEQ      # 4 sequences
H = 32            # query heads (total)
KVH = 8           # kv heads (total)
D = 128           # head size
DV = 128          # value head size
NCORES = 8
HPC = H // NCORES         # 4 query heads per core
NB = SEQ // 128           # 8 k-blocks per sequence
SCALE = 0.08838834764831845

F16 = mybir.dt.float16
F32 = mybir.dt.float32

_BUILD_CACHE = {}


def _build_nc():
    nc = bacc.Bacc("TRN2", target_bir_lowering=False, debug=False,
                   num_devices=NCORES)
    # host-pretransposed, fp16: qT[h*128+d, t], kT[d, t], v[t, dv]
    qt_dram = nc.dram_tensor("qT", [HPC * D, T], F16, kind="ExternalInput").ap()
    kt_dram = nc.dram_tensor("kT", [D, T], F16, kind="ExternalInput").ap()
    v_dram = nc.dram_tensor("v", [T, DV], F16, kind="ExternalInput").ap()
    # out_t[b*HPC + h, dv, q]  (transposed per-head output; host untransposes)
    out_dram = nc.dram_tensor("out_t", [B * HPC, DV, SEQ], F16,
                              kind="ExternalOutput").ap()

    with tile.TileContext(nc) as tc:
        with tc.tile_pool(name="consts", bufs=1) as consts, \
             tc.tile_pool(name="kv", bufs=2) as kv_pool, \
             tc.tile_pool(name="qts", bufs=5) as qt_pool, \
             tc.tile_pool(name="pt", bufs=4) as pt_pool, \
             tc.tile_pool(name="work", bufs=2) as work, \
             tc.tile_pool(name="pp_s", bufs=2, space="PSUM") as pp_s, \
             tc.tile_pool(name="pp_ol", bufs=4, space="PSUM") as pp_ol:

            ones_sb = consts.tile([128, 512], F16, tag="ones")
            nc.vector.memset(ones_sb[:], 1.0)

            # HAM clock warmup: a few dependency-free matmuls that run while
            # the first input chunks are still in flight, so the PE clock
            # gate starts ramping toward 2.4 GHz before real work issues.
            # Allocated from pp_ol: its slots are first recycled deep into
            # back(1), well after the last interleaved warmup fires.
            warm_ps = pp_ol.tile([128, 512], F32, tag="ps_ol")
            for _ in range(4):
                nc.tensor.matmul(warm_ps[:, 0:512], ones_sb[:, 0:128],
                                 ones_sb[:], start=True, stop=True,
                                 skip_group_check=True)

            per_b = {}   # b -> (kt, v_sb, [qt0..qt3])

            def emit_loads(b, chunks=1):
                """Load kt + qt0; chunks>1 splits them into kb-block chunks
                issued high-blocks-first so a descending-kb front can start
                after only the first chunk lands."""
                cols = slice(b * SEQ, (b + 1) * SEQ)
                kt = kv_pool.tile([128, NB, 128], F16, tag="kt")
                qt = qt_pool.tile([128, NB, 128], F16, tag="qt")
                step = NB // chunks
                for c in range(chunks - 1, -1, -1):
                    bs, be = c * step, (c + 1) * step
                    ccols = slice(b * SEQ + bs * 128, b * SEQ + be * 128)
                    nc.sync.dma_start(
                        kt[:, bs:be],
                        kt_dram[:, ccols].rearrange("d (nb t) -> d nb t", t=128))
                    nc.sync.dma_start(
                        qt[:, bs:be],
                        qt_dram[0:D, ccols].rearrange("d (nb t) -> d nb t", t=128))
                per_b[b] = (kt, None, [qt])

            def emit_late_loads(b, h):
                """After front(b, h) is emitted: pull in the next tensors."""
                cols = slice(b * SEQ, (b + 1) * SEQ)
                rows = slice(b * SEQ, (b + 1) * SEQ)
                kt, v_sb, qts = per_b[b]
                if h + 1 < HPC:
                    qt = qt_pool.tile([128, NB, 128], F16, tag="qt")
                    nc.sync.dma_start(
                        qt[:],
                        qt_dram[(h + 1) * D:(h + 2) * D, cols].rearrange(
                            "d (nb t) -> d nb t", t=128))
                    qts.append(qt)
                if h == 0:
                    v_sb = kv_pool.tile([128, NB, DV], F16, tag="v")
                    nc.sync.dma_start(
                        v_sb[:],
                        v_dram[rows, :].rearrange("(nb p) d -> p nb d", p=128))
                per_b[b] = (kt, v_sb, qts)

            def emit_front(b, h, descending=False):
                """QK matmuls + exp + causal mask -> dict kb -> P^T tile.

                descending=True runs kb 7..0 so the first matmuls only need
                the high kt/qt blocks (which chunked loads deliver first)."""
                kt, _, qts = per_b[b]
                qt = qts[h]
                pts = {}
                order = range(NB - 1, -1, -1) if descending else range(NB)
                for ikb, kb in enumerate(order):
                    if descending and ikb < 4:
                        # keep the HAM activity window dense while the later
                        # input chunks are still in flight (clock warmup)
                        for _ in range(3):
                            nc.tensor.matmul(
                                warm_ps[:, 0:512], ones_sb[:, 0:128],
                                ones_sb[:], start=True, stop=True,
                                skip_group_check=True)
                    ncols_t = SEQ - 128 * kb
                    pt = pt_pool.tile([128, ncols_t], F16, tag=f"pt{kb}")
                    # [128, 1024] psum tile (2 banks); kb>=4 uses cols 512:
                    ps = pp_s.tile([128, 1024], F32, tag="ps_s")
                    for qc in range(kb // 4, 2):
                        qs = max(128 * kb, 512 * qc)
                        qe = 512 * (qc + 1)
                        nc.tensor.matmul(
                            ps[:, qs:qe],
                            kt[:, kb, :],
                            qt[:, qs // 128:qe // 128, :],
                            start=True, stop=True, skip_group_check=True)
                    nc.scalar.activation(
                        pt[:], ps[:, 128 * kb:],
                        mybir.ActivationFunctionType.Exp, scale=SCALE)
                    # zero strictly-upper triangle of the diagonal block
                    nc.gpsimd.affine_select(
                        out=pt[:, 0:128], in_=pt[:, 0:128],
                        compare_op=mybir.AluOpType.is_ge,
                        fill=0.0, base=0,
                        pattern=[[1, 128]], channel_multiplier=-1)
                    pts[kb] = pt
                return pts

            def emit_back(b, h, pts):
                """PV + denominator matmuls, normalize, store."""
                _, v_sb, _ = per_b[b]
                out_sb = work.tile([128, SEQ], F16, tag="out_sb")
                for qc in range(2):
                    kbs = list(range(0, 4 * qc + 4))
                    ps_o = pp_ol.tile([128, 512], F32, tag="ps_ol")
                    ps_l = pp_ol.tile([128, 512], F32, tag="ps_ol")
                    for kb in kbs:
                        qs = max(128 * kb, 512 * qc)
                        qe = 512 * (qc + 1)
                        rhs = pts[kb][:, qs - 128 * kb:qe - 128 * kb]
                        flags = dict(start=(kb == 0), stop=(kb == kbs[-1]),
                                     skip_group_check=True)
                        # denominator group first so the DVE reciprocal can
                        # overlap the PV matmul stream
                        nc.tensor.matmul(
                            ps_l[:, qs - 512 * qc:512], ones_sb[:, 0:128], rhs,
                            **flags)
                        nc.tensor.matmul(
                            ps_o[:, qs - 512 * qc:512],
                            v_sb[:, kb, :], rhs, **flags)
                    rsb = work.tile([128, 512], F32, tag="rsb")
                    nc.vector.reciprocal_approx_fast(rsb[:], ps_l[:])
                    nc.vector.tensor_tensor(
                        out=out_sb[:, qc * 512:(qc + 1) * 512],
                        in0=ps_o[:], in1=rsb[:], op=mybir.AluOpType.mult)
                nc.sync.dma_start(out_dram[b * HPC + h], out_sb[:])

            # software-pipelined emission at depth 2: front(u+1) and
            # front(u+2) are emitted before back(u), so the exp/mask chain
            # of a unit has two units' worth of PE work to hide behind.
            units = [(b, h) for b in range(B) for h in range(HPC)]
            pending = deque()   # (b, h, pts)
            for b, h in units:
                if h == 0:
                    emit_loads(b, chunks=(2 if b == 0 else 1))
                pts = emit_front(b, h, descending=(b == 0 and h == 0))
                emit_late_loads(b, h)
                pending.append((b, h, pts))
                if len(pending) > 2:
                    emit_back(*pending.popleft())
            while pending:
                emit_back(*pending.popleft())

    nc.compile()
    return nc


# ---------------------------------------------------------------------------
# Phase B: grouped-exp restructure.
#
# Dropping the per-kb denominator ones-matmuls (24us of PE) in favour of a
# DVE collapse of the P^T tiles + one accumulating ones-matmul per q-half,
# and cutting the 128 per-kb exp ACTIVATEs (253ns fixed overhead each) to 48
# by packing each unit's causal score blocks into three [128,1536] PSUM
# tiles, exactly (zero padding):
#   g0: kb0 @ 0    (w 1024), kb4 @ 1024 (w 512)
#   g1: kb1 @ 0    (w 896),  kb3 @ 896  (w 640)
#   g2: kb2 @ 0    (w 768),  kb5 @ 768 (w 384), kb6 @ 1152 (w 256),
#       kb7 @ 1408 (w 128)
# Every matmul chunk is split at absolute 512-col PSUM bank boundaries and
# every chunk's q-start is 128-aligned. PSUM: 2x[128,1536] qk (6 banks) +
# [128,512] l (1 bank) + [128,512] out (1 bank) = 8 banks.
# ---------------------------------------------------------------------------

GROUPS = [
    [(0, 0), (4, 1024)],
    [(1, 0), (3, 896)],
    [(2, 0), (5, 768), (6, 1152), (7, 1408)],
]
# Unit-0 prologue table: group 0 = {kb4..7} needs only the high half of the
# kt/qt loads, so its exp can issue before the full load lands.
GROUPS0 = [
    [(4, 0), (5, 512), (6, 896), (7, 1152)],
    [(1, 0), (3, 896)],
    [(0, 0)],
    [(2, 0)],
]
LOAD_SPLITS0 = [(4, 8), (0, 4)]


def _loc(table):
    return {kb: (g, off) for g, lst in enumerate(table) for kb, off in lst}


def _qk_chunks(kb, off):
    """Sub-chunks (tile_c0, tile_c1, q0, q1) of kb's packed score region,
    split at absolute 512-col boundaries."""
    w = 1024 - 128 * kb
    out = []
    c = off
    while c < off + w:
        ce = min(off + w, (c // 512 + 1) * 512)
        q0 = 128 * kb + (c - off)
        out.append((c, ce, q0, q0 + (ce - c)))
        c = ce
    return out


def _pv_slices(qc, loc):
    """(kb, group, tile_c0, tile_c1, out_c0) for PV accumulation of q-half."""
    out = []
    for kb in range(8):
        g, off = loc[kb]
        q0 = max(128 * kb, 512 * qc)
        q1 = 512 * (qc + 1)
        if q0 >= q1:
            continue
        out.append((kb, g, off + (q0 - 128 * kb), off + (q1 - 128 * kb),
                    q0 - 512 * qc))
    return out


def _build_nc_b():
    nc = bacc.Bacc("TRN2", target_bir_lowering=False, debug=False,
                   num_devices=NCORES)
    qt_dram = nc.dram_tensor("qT", [HPC * D, T], F16, kind="ExternalInput").ap()
    kt_dram = nc.dram_tensor("kT", [D, T], F16, kind="ExternalInput").ap()
    v_dram = nc.dram_tensor("v", [T, DV], F16, kind="ExternalInput").ap()
    out_dram = nc.dram_tensor("out_t", [B * HPC, DV, SEQ], F16,
                              kind="ExternalOutput").ap()

    ADD = mybir.AluOpType.add
    MUL = mybir.AluOpType.mult
    EXP = mybir.ActivationFunctionType.Exp

    with tile.TileContext(nc) as tc:
        with tc.tile_pool(name="consts", bufs=1) as consts, \
             tc.tile_pool(name="kv", bufs=2) as kv_pool, \
             tc.tile_pool(name="qts", bufs=5) as qt_pool, \
             tc.tile_pool(name="pt", bufs=3) as pt_pool, \
             tc.tile_pool(name="ls", bufs=3) as ls_pool, \
             tc.tile_pool(name="work", bufs=2) as work, \
             tc.tile_pool(name="pp_qk", bufs=2, space="PSUM") as pp_qk, \
             tc.tile_pool(name="pp_l", bufs=1, space="PSUM") as pp_l, \
             tc.tile_pool(name="pp_o", bufs=1, space="PSUM") as pp_o:

            ones_sb = consts.tile([128, 512], F16, tag="ones")
            nc.vector.memset(ones_sb[:], 1.0)

            # HAM clock warmup (pp_l's slot is first reused by back1(0),
            # long after the last warmup matmul fires)
            warm_ps = pp_l.tile([128, 512], F32, tag="pl")
            for _ in range(10):
                nc.tensor.matmul(warm_ps[:], ones_sb[:, 0:128], ones_sb[:],
                                 start=True, stop=True, skip_group_check=True)

            per_b = {}

            def emit_loads(b, splits=((0, NB),)):
                """Each dma_start lands on its own DMA queue; descending
                block splits let the unit-0 per-group front start as soon as
                the first small chunk arrives."""
                kt = kv_pool.tile([128, NB, 128], F16, tag="kt")
                qt = qt_pool.tile([128, NB, 128], F16, tag="qt")
                for bs, be in splits:
                    ccols = slice(b * SEQ + bs * 128, b * SEQ + be * 128)
                    nc.sync.dma_start(
                        kt[:, bs:be],
                        kt_dram[:, ccols].rearrange("d (nb t) -> d nb t",
                                                    t=128))
                    nc.sync.dma_start(
                        qt[:, bs:be],
                        qt_dram[0:D, ccols].rearrange("d (nb t) -> d nb t",
                                                      t=128))
                per_b[b] = [kt, None, [qt]]

            def emit_late_loads(b, h):
                cols = slice(b * SEQ, (b + 1) * SEQ)
                rows = slice(b * SEQ, (b + 1) * SEQ)
                kt, v_sb, qts = per_b[b]
                if h + 1 < HPC:
                    qt = qt_pool.tile([128, NB, 128], F16, tag="qt")
                    nc.sync.dma_start(
                        qt[:],
                        qt_dram[(h + 1) * D:(h + 2) * D, cols].rearrange(
                            "d (nb t) -> d nb t", t=128))
                    qts.append(qt)
                if h == 0:
                    v_sb = kv_pool.tile([128, NB, DV], F16, tag="v")
                    nc.sync.dma_start(
                        v_sb[:],
                        v_dram[rows, :].rearrange("(nb p) d -> p nb d", p=128))
                per_b[b] = [kt, v_sb, qts]

            def emit_group(b, h, table, g, warm=False):
                """QK matmuls for one packed group + one wide exp + masks."""
                kt, _, qts = per_b[b]
                qt = qts[h]
                gps = pp_qk.tile([128, 1536], F32, tag="qk")
                for kb, off in table[g]:
                    if warm:
                        for _ in range(2):
                            nc.tensor.matmul(
                                warm_ps[:], ones_sb[:, 0:128], ones_sb[:],
                                start=True, stop=True, skip_group_check=True)
                    for c0, c1, q0, q1 in _qk_chunks(kb, off):
                        nc.tensor.matmul(
                            gps[:, c0:c1], kt[:, kb, :],
                            qt[:, q0 // 128:q1 // 128, :],
                            start=True, stop=True, skip_group_check=True)
                wmax = max(off + 1024 - 128 * kb for kb, off in table[g])
                gpt = pt_pool.tile([128, 1536], F16, tag=f"g{g}")
                nc.scalar.activation(gpt[:, 0:wmax], gps[:, 0:wmax], EXP,
                                     scale=SCALE)
                for kb, off in table[g]:
                    nc.gpsimd.affine_select(
                        out=gpt[:, off:off + 128], in_=gpt[:, off:off + 128],
                        compare_op=mybir.AluOpType.is_ge,
                        fill=0.0, base=0,
                        pattern=[[1, 128]], channel_multiplier=-1)
                return gpt

            def emit_collapse(loc, gpts):
                """lsum[k, q] = sum of P^T over all 8 kb."""
                ls = ls_pool.tile([128, 1024], F16, tag="ls")
                tt = nc.vector.tensor_tensor

                def sl(kb, a, bb):
                    g, off = loc[kb]
                    return gpts[g][:, off + a:off + bb]

                tt(out=ls[:, 128:1024], in0=sl(0, 128, 1024),
                   in1=sl(1, 0, 896), op=ADD)                     # kb0+kb1
                nc.vector.tensor_scalar_add(ls[:, 0:128], sl(0, 0, 128), 0.0)
                for kb in (2, 3, 5, 6, 7):
                    w = 1024 - 128 * kb
                    tt(out=ls[:, 128 * kb:1024], in0=ls[:, 128 * kb:1024],
                       in1=sl(kb, 0, w), op=ADD)
                return ls

            def _back_psum(alt):
                """pl/po PSUM for a back pass; alt=True carves them out of a
                pp_qk tile so the drain's two units don't serialize on the
                single pl/po banks."""
                if alt:
                    qk = pp_qk.tile([128, 1536], F32, tag="qk")
                    return qk[:, 0:512], qk[:, 512:1024]
                pl = pp_l.tile([128, 512], F32, tag="pl")
                po = pp_o.tile([128, 512], F32, tag="po")
                return pl[:], po[:]

            def emit_back1(rec, alt=False):
                """qc0: denominator ones-matmul, reciprocal, PV, normalize."""
                b, h, loc, gpts, ls = rec
                _, v_sb, _ = per_b[b]
                pl, po = _back_psum(alt)
                nc.tensor.matmul(pl, ones_sb[:, 0:128], ls[:, 0:512],
                                 start=True, stop=True, skip_group_check=True)
                rs = work.tile([128, 512], F32, tag="rs")
                nc.vector.reciprocal_approx_fast(rs[:], pl)
                sl = _pv_slices(0, loc)
                for i, (kb, g, c0, c1, oc) in enumerate(sl):
                    nc.tensor.matmul(
                        po[:, oc:512], v_sb[:, kb, :], gpts[g][:, c0:c1],
                        start=(i == 0), stop=(i == len(sl) - 1),
                        skip_group_check=True)
                os = work.tile([128, SEQ], F16, tag="out_sb")
                nc.vector.tensor_tensor(out=os[:, 0:512], in0=po,
                                        in1=rs[:], op=MUL)
                nc.sync.dma_start(out_dram[b * HPC + h][:, 0:512],
                                  os[:, 0:512])
                rec.append(os)

            def emit_back2(rec, alt=False):
                """qc1 (kb4's denominator joins via PSUM accumulation)."""
                b, h, loc, gpts, ls, os = rec
                _, v_sb, _ = per_b[b]
                g4, off4 = loc[4]
                pl, po = _back_psum(alt)
                nc.tensor.matmul(pl, ones_sb[:, 0:128], ls[:, 512:1024],
                                 start=True, stop=False, skip_group_check=True)
                nc.tensor.matmul(pl, ones_sb[:, 0:128],
                                 gpts[g4][:, off4:off4 + 512],
                                 start=False, stop=True, skip_group_check=True)
                rs = work.tile([128, 512], F32, tag="rs")
                nc.vector.reciprocal_approx_fast(rs[:], pl)
                sl = _pv_slices(1, loc)
                for i, (kb, g, c0, c1, oc) in enumerate(sl):
                    nc.tensor.matmul(
                        po[:, oc:512], v_sb[:, kb, :], gpts[g][:, c0:c1],
                        start=(i == 0), stop=(i == len(sl) - 1),
                        skip_group_check=True)
                nc.vector.tensor_tensor(out=os[:, 512:1024], in0=po,
                                        in1=rs[:], op=MUL)
                nc.sync.dma_start(out_dram[b * HPC + h][:, 512:1024],
                                  os[:, 512:1024])

            # Software pipeline at depth 2: back1(u-2) lands between the
            # current unit's groups, back2(u-2) after its collapse, keeping
            # each unit's reciprocal two units ahead of the PE work that
            # consumes it in the in-order DVE queue.
            units = [(b, h) for b in range(B) for h in range(HPC)]
            pending = deque()
            for b, h in units:
                if h == 0:
                    emit_loads(b, splits=(LOAD_SPLITS0 if b == 0
                                          else ((0, NB),)))
                warm = (b == 0 and h == 0)
                table = GROUPS0 if warm else GROUPS
                loc = _loc(table)
                gpts = [emit_group(b, h, table, 0, warm=warm),
                        emit_group(b, h, table, 1)]
                if len(pending) >= 2:
                    emit_back1(pending[0])
                for g in range(2, len(table)):
                    gpts.append(emit_group(b, h, table, g))
                ls = emit_collapse(loc, gpts)
                emit_late_loads(b, h)
                if len(pending) >= 2:
                    emit_back2(pending.popleft())
                pending.append([b, h, loc, gpts, ls])
            # drain: interleave the two remaining units' back chains, the
            # second on pp_qk-carved PSUM so they don't serialize on pl/po
            ra = pending.popleft()
            rb = pending.popleft()
            emit_back1(ra)
            emit_back1(rb, alt=True)
            emit_back2(ra)
            emit_back2(rb, alt=True)

    nc.compile()
    return nc


def run_sharded(query, key, value, trace=False):
    """Shard over 8 cores, run the bass kernel, unshard. Returns
    (out [T, H*DV] fp32, BassKernelResults)."""
    query = np.asarray(query, dtype=np.float32)
    key = np.asarray(key, dtype=np.float32)
    value = np.asarray(value, dtype=np.float32)

    ver = os.environ.get("ATTN_KERNEL_VER", "b")
    if ("nc", ver) not in _BUILD_CACHE:
        _BUILD_CACHE[("nc", ver)] = _build_nc_b() if ver == "b" else _build_nc()
    nc = _BUILD_CACHE[("nc", ver)]

    # host layout glue: cast to fp16, then transpose to [d, t]
    qT = np.ascontiguousarray(query.astype(np.float16).T)   # [H*D, T]
    kT = np.ascontiguousarray(key.astype(np.float16).T)     # [KVH*D, T]
    v16 = np.ascontiguousarray(value.astype(np.float16))    # [T, KVH*DV]

    in_maps = []
    for c in range(NCORES):
        in_maps.append({
            "qT": np.ascontiguousarray(qT[c * HPC * D:(c + 1) * HPC * D]),
            "kT": np.ascontiguousarray(kT[c * D:(c + 1) * D]),
            "v": np.ascontiguousarray(v16[:, c * DV:(c + 1) * DV]),
        })

    res = bass_utils.run_bass_kernel_spmd(
        nc, in_maps, core_ids=list(range(NCORES)), trace=trace)

    outs = []
    for c in range(NCORES):
        ot = res.results[c]["out_t"].astype(np.float32)  # [B*HPC, DV, SEQ] f16
        o = ot.reshape(B, HPC, DV, SEQ).transpose(0, 3, 1, 2).reshape(T, HPC * DV)
        outs.append(o)
    return np.concatenate(outs, axis=1), res


def kernel(query, key, value, seq_len=1024, **_unused):
    assert int(seq_len) == SEQ, f"kernel hardcodes seq_len={SEQ}, got {seq_len}"
    out, _ = run_sharded(query, key, value, trace=False)
    return out

